# revision 23
# baseline (speedup 1.0000x reference)
"""Multi-head self-attention (B=4, S=2048, D=1024, H=16) on 8 trn2 NeuronCores.

Sharding: batch (4) x head-group (2 groups of 8 heads) -> 8 cores.
Each core computes, for its (batch b, head-group hg):
  Q'^T = (wq_l/8) @ x_b^T            [512, 2048]   (1/sqrt(dk) folded into wq)
  K^T  = wk_l @ x_b^T                [512, 2048]
  V    = x_b @ wv_l^T                [2048, 512]
  per head h (8 local, dk=64), in transposed layout (keys on partitions):
    scoresT[k, q] = K_h @ Q'_h^T     (no max-subtraction: scores ~ N(0,4), exp
                                      of |s|<~12 is safe in fp32/bf16)
    expT = exp(scoresT)              (ScalarE, PSUM->SBUF bf16)
    unnormT[c, q] = V_h^T @ expT     (PE, accumulated over key tiles)
    Z[q] = ones^T @ expT             (PE colsum quads, same accumulation)
    attnT = unnormT / Z              (reciprocal once + DMA partition
                                      broadcast via DRAM + DVE muls)
  out_partial = attnT^T @ wo_l^T     [2048, 1024]  (row-parallel wo)
Host sums the two partials per batch (the "all-reduce" of row-parallel wo).

Pipeline shape (per core): the 256 exps (ScalarE, ~1.1us each) are the
theoretical floor; scores/V/colsum matmuls are scheduled so the exp chain
never stalls (V/colsum lag one key-tile behind scores, g1-3 Q/K projection
chunks are injected into the PE's slack), everything else is off-path.
"""

import ml_dtypes
import numpy as np

import bass_rust
import concourse.bass as bass
import concourse.mybir as mybir
import concourse.tile as tile

# ---------------------------------------------------------------- constants
S = 2048          # sequence length
DM = 1024         # model dim
DL = 512          # local (per-core) head dims = 8 heads * 64
DK = 64           # head dim
P = 128
NKT = S // P      # 16 key tiles
NG = DL // P      # 4 head-pairs (c-tiles / dq-tiles)
KD = DM // P      # 8 contraction tiles for projections
NSC = S // 512    # 4 s-chunks for projections
F32 = mybir.dt.float32
BF16 = mybir.dt.bfloat16
BF16_NP = ml_dtypes.bfloat16

N_CORES = 8
CORE_IDS = list(range(N_CORES))


# ------------------------------------------------- walrus sync-wait workaround
def _split_sync_waits(nc, limit=1):
    """This toolchain's walrus codegen rejects instructions carrying more than
    one sync-wait command.  Move excess waits onto dedicated same-engine nops
    inserted immediately before the instruction (sequential waits on the same
    engine queue are semantically identical to multiple waits on one inst)."""
    fn = nc.m.functions[0]
    snapshots = [(bb, list(bb.instructions)) for bb in fn.blocks]
    plans = []
    for _bb, insts in snapshots:
        plan = {}
        for idx, inst in enumerate(insts):
            si = inst.sync_info
            waits = list(si.on_wait) if si and si.on_wait else []
            if len(waits) > limit:
                pre, keep = waits[:-limit], waits[-limit:]
                nops = []
                for w in pre:
                    ni = nc.engines[inst.engine].nop(nofuse=True, hint="wsplit").ins
                    ni.sync_info = bass_rust.SyncInfo(on_wait=[w], on_update=[])
                    nops.append(ni)
                si.on_wait = keep
                plan[idx] = nops
        plans.append(plan)
    # Rebuild every block from its pre-pass snapshot plus insertions; this also
    # drops the fresh nops from wherever bass appended them at creation time.
    for (bb, insts), plan in zip(snapshots, plans):
        out = []
        for idx, inst in enumerate(insts):
            out.extend(plan.get(idx, ()))
            out.append(inst)
        bb.instructions = out


# ---------------------------------------------------------------- the program
def build_nc():
    """Build the SPMD per-core Bass program (identical on all 8 cores)."""
    nc = bass.Bass()

    xT = nc.declare_dram_parameter("xT", [DM, S], BF16, isOutput=False)
    wqT = nc.declare_dram_parameter("wqT", [DM, DL], BF16, isOutput=False)
    wkT = nc.declare_dram_parameter("wkT", [DM, DL], BF16, isOutput=False)
    wvT = nc.declare_dram_parameter("wvT", [DM, DL], BF16, isOutput=False)
    woT = nc.declare_dram_parameter("woT", [DL, DM], BF16, isOutput=False)
    out = nc.declare_dram_parameter("out", [S, DM], F32, isOutput=True)

    with tile.TileContext(nc) as tc:
        with (
            tc.tile_pool(name="big", bufs=1) as big,
            tc.tile_pool(name="expT", bufs=8) as expp,
            tc.tile_pool(name="rc", bufs=2) as rcp,
            tc.tile_pool(name="outsb", bufs=3) as outp,
            tc.tile_pool(name="dram", bufs=2, space="DRAM") as dramp,
            tc.tile_pool(name="ps", bufs=2, space="PSUM") as psp,
            tc.tile_pool(name="av", bufs=3, space="PSUM") as avp,
            tc.tile_pool(name="cs", bufs=1, space="PSUM") as csp,
        ):
            # ---------------- load everything from DRAM (weights first; x
            # split across several dma_starts so multiple queues run it)
            w_sb = {}
            for name, dram in (("wk", wkT), ("wq", wqT), ("wv", wvT)):
                w_sb[name] = big.tile([P, KD, DL], BF16, tag=name, name=name)
                nc.sync.dma_start(
                    w_sb[name][:], dram.rearrange("(kd p) m -> p kd m", p=P)
                )
            xT_sb = big.tile([P, KD, S], BF16, tag="xT")
            xT_r = xT.rearrange("(kd p) s -> p kd s", p=P)
            for kd2 in range(0, KD, 2):
                nc.sync.dma_start(
                    xT_sb[:, kd2 : kd2 + 2, :], xT_r[:, kd2 : kd2 + 2, :]
                )
            woT_sb = big.tile([P, NG, DM], BF16, tag="wo")
            nc.sync.dma_start(woT_sb[:], woT.rearrange("(ct p) o -> p ct o", p=P))

            # ---------------- constants
            ones_bf = big.tile([P, 1], BF16, tag="ones")
            nc.vector.memset(ones_bf[:], 1.0)

            # persistent activation tensors
            QT = [big.tile([P, S], BF16, tag=f"QT{g}", name=f"QT{g}") for g in range(NG)]
            KT = [big.tile([P, S], BF16, tag=f"KT{g}", name=f"KT{g}") for g in range(NG)]
            V_st = [big.tile([P, 8, DK + 1], BF16, tag=f"V{st}", name=f"V{st}") for st in range(NKT)]
            attn = [big.tile([P, S], BF16, tag=f"attn{g}", name=f"attn{g}") for g in range(NG)]

            # ---------------- projections
            def proj_qk_chunk(dst, w, g, sc, pool, tag):
                ps = pool.tile([P, 512], F32, tag=tag, name="projch")
                for kd in range(KD):
                    nc.tensor.matmul(
                        ps[:],
                        lhsT=w[:, kd, g * P : (g + 1) * P],
                        rhs=xT_sb[:, kd, sc * 512 : (sc + 1) * 512],
                        start=(kd == 0),
                        stop=(kd == KD - 1),
                    )
                nc.vector.tensor_copy(out=dst[:, sc * 512 : (sc + 1) * 512], in_=ps[:])

            def proj_qk(dst, w, g):
                """dst[:, :] (tile g) = rows [128g,128g+128) of W' @ x^T."""
                for sc in range(NSC):
                    ps = psp.tile([P, 512], F32, tag="ps")
                    for kd in range(KD):
                        nc.tensor.matmul(
                            ps[:],
                            lhsT=w[:, kd, g * P : (g + 1) * P],
                            rhs=xT_sb[:, kd, sc * 512 : (sc + 1) * 512],
                            start=(kd == 0),
                            stop=(kd == KD - 1),
                        )
                    nc.vector.tensor_copy(
                        out=dst[:, sc * 512 : (sc + 1) * 512], in_=ps[:]
                    )

            def proj_v(st):
                ps = psp.tile([P, 512], F32, tag="ps")
                for kd in range(KD):
                    nc.tensor.matmul(
                        ps[:],
                        lhsT=xT_sb[:, kd, st * P : (st + 1) * P],
                        rhs=w_sb["wv"][:, kd, :],
                        start=(kd == 0),
                        stop=(kd == KD - 1),
                    )
                nc.vector.tensor_copy(
                    out=V_st[st][:, :, 0:DK],
                    in_=ps.rearrange("p (h c) -> p h c", c=DK),
                )

            # g=0 first so attention can start early; the g1-3 Q/K
            # projections are deferred into the attention stream's PE slack
            proj_qk(KT[0], w_sb["wk"], 0)
            proj_qk(QT[0], w_sb["wq"], 0)
            for st in range(NKT):
                proj_v(st)
            def wo_chunk(st, ob, pool, tag):
                ps = pool.tile([P, 512], F32, tag=tag, name="wochunk")
                for ct in range(NG):
                    nc.tensor.matmul(
                        ps[:],
                        lhsT=attn[ct][:, st * P : (st + 1) * P],
                        rhs=woT_sb[:, ct, ob * 512 : (ob + 1) * 512],
                        start=(ct == 0),
                        stop=(ct == NG - 1),
                    )
                ot = outp.tile([P, 512], F32, tag="out")
                nc.vector.tensor_copy(out=ot[:], in_=ps[:])
                nc.sync.dma_start(
                    out[st * P : (st + 1) * P, ob * 512 : (ob + 1) * 512], ot[:]
                )

            deferred_proj = [
                (dst[g], w, g, sc)
                for g in range(1, NG)
                for dst, w in ((KT, w_sb["wk"]), (QT, w_sb["wq"]))
                for sc in range(NSC)
            ]

            # ---------------- attention
            class AttnBlock:
                """Heads A=2g, B=2g+1; query half qh (1024 queries).

                scoresT/exp are ACT-paced.  V and colsum matmuls lag one kt
                behind (carried across block boundaries by the driver loop) so
                both heads' exp tiles are ready together, letting adjacently
                issued matmuls with disjoint array tile positions (V: col
                groups 0-1 vs 2-3; colsums: 32-strips 0/32/64/96) run
                concurrently on the PE.  vt accumulates A in partitions 0-63
                and B in 64-127 of one bank (memset + start=False keeps the
                interleaved accumulation groups from clearing each other).
                Normalization runs entirely off the critical path.
                """

                def __init__(self, g, qh):
                    self.g, self.qoff = g, qh * 1024
                    self.vt = [
                        avp.tile([P, 512], F32, tag="av", name=f"vt{qb}")
                        for qb in range(2)
                    ]
                    self.cs = csp.tile([P, 512], F32, tag="cs")
                    for t in self.vt:
                        nc.vector.memset(t[:], 0.0)
                    nc.vector.memset(self.cs[:], 0.0)
                    self.ets = {}

                def emit_scores_exp(self, kt):
                    g, qoff = self.g, self.qoff
                    for hp, pb in ((0, 0), (1, 64)):
                        ps_s = psp.tile([P, 1024], F32, tag="ps", name=f"ps_s{hp}")
                        for qb in range(2):
                            nc.tensor.matmul(
                                ps_s[:, qb * 512 : (qb + 1) * 512],
                                lhsT=KT[g][pb : pb + 64, kt * P : (kt + 1) * P],
                                rhs=QT[g][
                                    pb : pb + 64,
                                    qoff + qb * 512 : qoff + (qb + 1) * 512,
                                ],
                                start=True,
                                stop=True,
                            )
                        et = expp.tile([P, 1024], BF16, tag="expT", name=f"et{hp}")
                        nc.scalar.activation(
                            et[:], ps_s[:], mybir.ActivationFunctionType.Exp
                        )
                        self.ets[(kt, hp)] = et

                def emit_v_cs(self, kt):
                    g = self.g
                    last = kt == NKT - 1
                    et = {hp: self.ets.pop((kt, hp)) for hp in (0, 1)}
                    for qb in range(2):
                        for hp, pb in ((0, 0), (1, 64)):
                            nc.tensor.matmul(
                                self.vt[qb][pb : pb + 64, :],
                                lhsT=V_st[kt][:, 2 * g + hp, 0:DK],
                                rhs=et[hp][:, qb * 512 : (qb + 1) * 512],
                                start=False,
                                stop=last,
                                skip_group_check=True,
                                tile_position=(0, pb),
                            )
                    for hp in (0, 1):
                        for qb in range(2):
                            cp = 64 * hp + 32 * qb
                            nc.tensor.matmul(
                                self.cs[cp : cp + 1, :],
                                lhsT=ones_bf[:],
                                rhs=et[hp][:, qb * 512 : (qb + 1) * 512],
                                start=False,
                                stop=last,
                                skip_group_check=True,
                                tile_position=(0, cp),
                            )
                    if last:
                        self.emit_norm()

                def emit_norm(self):
                    g, qoff = self.g, self.qoff
                    un = [
                        rcp.tile([P, 512], F32, tag=f"un{qb}", name=f"un{qb}")
                        for qb in range(2)
                    ]
                    for qb in range(2):
                        nc.vector.tensor_copy(out=un[qb][:], in_=self.vt[qb][:])
                    cs_sb = rcp.tile([P, 512], F32, tag="cs_sb")
                    nc.vector.tensor_copy(out=cs_sb[:], in_=self.cs[:])
                    # reciprocal of the whole tile; junk rows give inf but only
                    # the four real rows {0,32,64,96} are read (by the DMA)
                    cs_rc = rcp.tile([P, 512], F32, tag="cs_rc")
                    nc.vector.reciprocal(cs_rc[:], cs_sb[:])
                    zd = dramp.tile([4, 512], F32, name="zd")
                    # zd rows: 0=(A,qb0) 1=(A,qb1) 2=(B,qb0) 3=(B,qb1)
                    nc.sync.dma_start(zd[:], cs_rc[0:128:32, :])
                    for qb in range(2):
                        rcb = rcp.tile(
                            [P, 512], F32, tag=f"rcb{qb}", name=f"rcb{qb}"
                        )
                        nc.sync.dma_start(
                            rcb[0:64, :], zd[qb, None, :].to_broadcast([64, 512])
                        )
                        nc.sync.dma_start(
                            rcb[64:128, :],
                            zd[qb + 2, None, :].to_broadcast([64, 512]),
                        )
                        for pb in (0, 64):
                            nc.vector.tensor_mul(
                                out=attn[g][
                                    pb : pb + 64,
                                    qoff + qb * 512 : qoff + (qb + 1) * 512,
                                ],
                                in0=un[qb][pb : pb + 64, :],
                                in1=rcb[pb : pb + 64, :],
                            )

            deferred_wo = []
            pending = None
            gkt = 0
            for g in range(NG):
                for qh in range(2):
                    blk = AttnBlock(g, qh)
                    for kt in range(NKT):
                        blk.emit_scores_exp(kt)
                        if pending is not None:
                            pending[0].emit_v_cs(pending[1])
                        pending = (blk, kt)
                        if gkt % 4 == 2 and deferred_proj:
                            proj_qk_chunk(*deferred_proj.pop(0), avp, "av")
                        if g == NG - 1 and qh == 1 and deferred_wo:
                            wo_chunk(*deferred_wo.pop(0), avp, "av")
                        gkt += 1
            pending[0].emit_v_cs(pending[1])
            assert not deferred_proj and not deferred_wo

            # ---------------- output projection (second half; the first half
            # was injected into the last attention block's PE slack)
            for st in range(NKT):
                for ob in range(2):
                    wo_chunk(st, ob, psp, "ps")

    _split_sync_waits(nc)
    return nc


_NC = None


def _get_nc():
    global _NC
    if _NC is None:
        _NC = build_nc()
    return _NC


# ---------------------------------------------------------------- host side
def make_in_maps(x, wq, wk, wv, wo):
    x = np.asarray(x, dtype=np.float32)
    wq = np.asarray(wq, dtype=np.float32)
    wk = np.asarray(wk, dtype=np.float32)
    wv = np.asarray(wv, dtype=np.float32)
    wo = np.asarray(wo, dtype=np.float32)
    in_maps = []
    for c in range(N_CORES):
        b, hg = c // 2, c % 2
        sl = slice(hg * DL, (hg + 1) * DL)
        xTc = np.ascontiguousarray(x[b].T).astype(BF16_NP)
        wqTc = np.ascontiguousarray((wq[sl] / 8.0).T).astype(BF16_NP)
        wkTc = np.ascontiguousarray(wk[sl].T).astype(BF16_NP)
        wvTc = np.ascontiguousarray(wv[sl].T).astype(BF16_NP)
        woTc = np.ascontiguousarray(wo[:, sl].T).astype(BF16_NP)
        in_maps.append(
            {"xT": xTc, "wqT": wqTc, "wkT": wkTc, "wvT": wvTc, "woT": woTc}
        )
    return in_maps


def gather(results):
    out = np.zeros((4, S, DM), dtype=np.float32)
    for c in range(N_CORES):
        out[c // 2] += results[c]["out"]
    return out


def kernel(x, wq, wk, wv, wo):
    from concourse.bass_utils import run_bass_kernel_spmd

    nc = _get_nc()
    in_maps = make_in_maps(x, wq, wk, wv, wo)
    res = run_bass_kernel_spmd(nc, in_maps, CORE_IDS)
    return gather(res.results)


# revision 24
# speedup vs baseline: 1.0007x; 1.0007x over previous
"""Multi-head self-attention (B=4, S=2048, D=1024, H=16) on 8 trn2 NeuronCores.

Sharding: batch (4) x head-group (2 groups of 8 heads) -> 8 cores.
Each core computes, for its (batch b, head-group hg):
  Q'^T = (wq_l/8) @ x_b^T            [512, 2048]   (1/sqrt(dk) folded into wq)
  K^T  = wk_l @ x_b^T                [512, 2048]
  V    = x_b @ wv_l^T                [2048, 512]
  per head h (8 local, dk=64), in transposed layout (keys on partitions):
    scoresT[k, q] = K_h @ Q'_h^T     (no max-subtraction: scores ~ N(0,4), exp
                                      of |s|<~12 is safe in fp32/bf16)
    expT = exp(scoresT)              (ScalarE, PSUM->SBUF bf16)
    unnormT[c, q] = V_h^T @ expT     (PE, accumulated over key tiles)
    Z[q] = ones^T @ expT             (PE colsum quads, same accumulation)
    attnT = unnormT / Z              (reciprocal once + DMA partition
                                      broadcast via DRAM + DVE muls)
  out_partial = attnT^T @ wo_l^T     [2048, 1024]  (row-parallel wo)
Host sums the two partials per batch (the "all-reduce" of row-parallel wo).

Pipeline shape (per core): the 256 exps (ScalarE, ~1.1us each) are the
theoretical floor; scores/V/colsum matmuls are scheduled so the exp chain
never stalls (V/colsum lag one key-tile behind scores, g1-3 Q/K projection
chunks are injected into the PE's slack), everything else is off-path.
"""

import ml_dtypes
import numpy as np

import bass_rust
import concourse.bass as bass
import concourse.mybir as mybir
import concourse.tile as tile

# ---------------------------------------------------------------- constants
S = 2048          # sequence length
DM = 1024         # model dim
DL = 512          # local (per-core) head dims = 8 heads * 64
DK = 64           # head dim
P = 128
NKT = S // P      # 16 key tiles
NG = DL // P      # 4 head-pairs (c-tiles / dq-tiles)
KD = DM // P      # 8 contraction tiles for projections
NSC = S // 512    # 4 s-chunks for projections
F32 = mybir.dt.float32
BF16 = mybir.dt.bfloat16
BF16_NP = ml_dtypes.bfloat16

N_CORES = 8
CORE_IDS = list(range(N_CORES))


# ------------------------------------------------- walrus sync-wait workaround
def _split_sync_waits(nc, limit=1):
    """This toolchain's walrus codegen rejects instructions carrying more than
    one sync-wait command.  Move excess waits onto dedicated same-engine nops
    inserted immediately before the instruction (sequential waits on the same
    engine queue are semantically identical to multiple waits on one inst)."""
    fn = nc.m.functions[0]
    snapshots = [(bb, list(bb.instructions)) for bb in fn.blocks]
    plans = []
    for _bb, insts in snapshots:
        plan = {}
        for idx, inst in enumerate(insts):
            si = inst.sync_info
            waits = list(si.on_wait) if si and si.on_wait else []
            if len(waits) > limit:
                pre, keep = waits[:-limit], waits[-limit:]
                nops = []
                for w in pre:
                    ni = nc.engines[inst.engine].nop(nofuse=True, hint="wsplit").ins
                    ni.sync_info = bass_rust.SyncInfo(on_wait=[w], on_update=[])
                    nops.append(ni)
                si.on_wait = keep
                plan[idx] = nops
        plans.append(plan)
    # Rebuild every block from its pre-pass snapshot plus insertions; this also
    # drops the fresh nops from wherever bass appended them at creation time.
    for (bb, insts), plan in zip(snapshots, plans):
        out = []
        for idx, inst in enumerate(insts):
            out.extend(plan.get(idx, ()))
            out.append(inst)
        bb.instructions = out


# ---------------------------------------------------------------- the program
def build_nc():
    """Build the SPMD per-core Bass program (identical on all 8 cores)."""
    nc = bass.Bass()

    xT = nc.declare_dram_parameter("xT", [DM, S], BF16, isOutput=False)
    wqT = nc.declare_dram_parameter("wqT", [DM, DL], BF16, isOutput=False)
    wkT = nc.declare_dram_parameter("wkT", [DM, DL], BF16, isOutput=False)
    wvT = nc.declare_dram_parameter("wvT", [DM, DL], BF16, isOutput=False)
    woT = nc.declare_dram_parameter("woT", [DL, DM], BF16, isOutput=False)
    out = nc.declare_dram_parameter("out", [S, DM], F32, isOutput=True)

    with tile.TileContext(nc) as tc:
        with (
            tc.tile_pool(name="big", bufs=1) as big,
            tc.tile_pool(name="expT", bufs=8) as expp,
            tc.tile_pool(name="rc", bufs=2) as rcp,
            tc.tile_pool(name="outsb", bufs=3) as outp,
            tc.tile_pool(name="dram", bufs=2, space="DRAM") as dramp,
            tc.tile_pool(name="ps", bufs=2, space="PSUM") as psp,
            tc.tile_pool(name="av", bufs=3, space="PSUM") as avp,
            tc.tile_pool(name="cs", bufs=1, space="PSUM") as csp,
        ):
            # ---------------- load everything from DRAM (weights first; x
            # split across several dma_starts so multiple queues run it)
            w_sb = {}
            for name, dram in (("wk", wkT), ("wq", wqT), ("wv", wvT)):
                w_sb[name] = big.tile([P, KD, DL], BF16, tag=name, name=name)
                nc.sync.dma_start(
                    w_sb[name][:], dram.rearrange("(kd p) m -> p kd m", p=P)
                )
            xT_sb = big.tile([P, KD, S], BF16, tag="xT")
            xT_r = xT.rearrange("(kd p) s -> p kd s", p=P)
            for kd2 in range(0, KD, 2):
                nc.sync.dma_start(
                    xT_sb[:, kd2 : kd2 + 2, :], xT_r[:, kd2 : kd2 + 2, :]
                )
            woT_sb = big.tile([P, NG, DM], BF16, tag="wo")
            nc.sync.dma_start(woT_sb[:], woT.rearrange("(ct p) o -> p ct o", p=P))

            # ---------------- constants
            ones_bf = big.tile([P, 1], BF16, tag="ones")
            nc.vector.memset(ones_bf[:], 1.0)

            # persistent activation tensors
            QT = [big.tile([P, S], BF16, tag=f"QT{g}", name=f"QT{g}") for g in range(NG)]
            KT = [big.tile([P, S], BF16, tag=f"KT{g}", name=f"KT{g}") for g in range(NG)]
            V_st = [big.tile([P, 8, DK + 1], BF16, tag=f"V{st}", name=f"V{st}") for st in range(NKT)]
            attn = [big.tile([P, S], BF16, tag=f"attn{g}", name=f"attn{g}") for g in range(NG)]

            # ---------------- projections
            def proj_qk_chunk(dst, w, g, sc, pool, tag):
                ps = pool.tile([P, 512], F32, tag=tag, name="projch")
                for kd in range(KD):
                    nc.tensor.matmul(
                        ps[:],
                        lhsT=w[:, kd, g * P : (g + 1) * P],
                        rhs=xT_sb[:, kd, sc * 512 : (sc + 1) * 512],
                        start=(kd == 0),
                        stop=(kd == KD - 1),
                    )
                nc.vector.tensor_copy(out=dst[:, sc * 512 : (sc + 1) * 512], in_=ps[:])

            def proj_qk(dst, w, g):
                """dst[:, :] (tile g) = rows [128g,128g+128) of W' @ x^T."""
                for sc in range(NSC):
                    ps = psp.tile([P, 512], F32, tag="ps")
                    for kd in range(KD):
                        nc.tensor.matmul(
                            ps[:],
                            lhsT=w[:, kd, g * P : (g + 1) * P],
                            rhs=xT_sb[:, kd, sc * 512 : (sc + 1) * 512],
                            start=(kd == 0),
                            stop=(kd == KD - 1),
                        )
                    nc.vector.tensor_copy(
                        out=dst[:, sc * 512 : (sc + 1) * 512], in_=ps[:]
                    )

            def proj_v(st):
                ps = psp.tile([P, 512], F32, tag="ps")
                for kd in range(KD):
                    nc.tensor.matmul(
                        ps[:],
                        lhsT=xT_sb[:, kd, st * P : (st + 1) * P],
                        rhs=w_sb["wv"][:, kd, :],
                        start=(kd == 0),
                        stop=(kd == KD - 1),
                    )
                nc.vector.tensor_copy(
                    out=V_st[st][:, :, 0:DK],
                    in_=ps.rearrange("p (h c) -> p h c", c=DK),
                )

            # g=0 first so attention can start early; the g1-3 Q/K
            # projections are deferred into the attention stream's PE slack
            proj_qk(KT[0], w_sb["wk"], 0)
            proj_qk(QT[0], w_sb["wq"], 0)
            for st in range(NKT):
                proj_v(st)
            def wo_chunk(st, ob, pool, tag):
                ps = pool.tile([P, 512], F32, tag=tag, name="wochunk")
                for ct in range(NG):
                    nc.tensor.matmul(
                        ps[:],
                        lhsT=attn[ct][:, st * P : (st + 1) * P],
                        rhs=woT_sb[:, ct, ob * 512 : (ob + 1) * 512],
                        start=(ct == 0),
                        stop=(ct == NG - 1),
                    )
                ot = outp.tile([P, 512], F32, tag="out")
                nc.vector.tensor_copy(out=ot[:], in_=ps[:])
                nc.sync.dma_start(
                    out[st * P : (st + 1) * P, ob * 512 : (ob + 1) * 512], ot[:]
                )

            class ProjInjector:
                """Emits deferred projection chunks two matmuls at a time so
                the injected PE work stays below the exp chain's slack and
                never lumps into an ACT stall."""

                def __init__(self, items):
                    self.items = list(items)
                    self.cur = None
                    self.ps = None
                    self.kd = 0

                def tick(self):
                    if self.cur is None:
                        if not self.items:
                            return
                        self.cur = self.items.pop(0)
                        self.kd = 0
                        self.ps = avp.tile([P, 512], F32, tag="av", name="projch")
                    dst, w, g, sc = self.cur
                    for _ in range(2):
                        nc.tensor.matmul(
                            self.ps[:],
                            lhsT=w[:, self.kd, g * P : (g + 1) * P],
                            rhs=xT_sb[:, self.kd, sc * 512 : (sc + 1) * 512],
                            start=(self.kd == 0),
                            stop=(self.kd == KD - 1),
                        )
                        self.kd += 1
                    if self.kd == KD:
                        nc.vector.tensor_copy(
                            out=dst[:, sc * 512 : (sc + 1) * 512], in_=self.ps[:]
                        )
                        self.cur = None

                def drain(self):
                    while self.items or self.cur is not None:
                        self.tick()

            injector = ProjInjector(
                (dst[g], w, g, sc)
                for g in range(1, NG)
                for dst, w in ((KT, w_sb["wk"]), (QT, w_sb["wq"]))
                for sc in range(NSC)
            )

            # ---------------- attention
            class AttnBlock:
                """Heads A=2g, B=2g+1; query half qh (1024 queries).

                scoresT/exp are ACT-paced.  V and colsum matmuls lag one kt
                behind (carried across block boundaries by the driver loop) so
                both heads' exp tiles are ready together, letting adjacently
                issued matmuls with disjoint array tile positions (V: col
                groups 0-1 vs 2-3; colsums: 32-strips 0/32/64/96) run
                concurrently on the PE.  vt accumulates A in partitions 0-63
                and B in 64-127 of one bank (memset + start=False keeps the
                interleaved accumulation groups from clearing each other).
                Normalization runs entirely off the critical path.
                """

                def __init__(self, g, qh):
                    self.g, self.qoff = g, qh * 1024
                    self.vt = [
                        avp.tile([P, 512], F32, tag="av", name=f"vt{qb}")
                        for qb in range(2)
                    ]
                    self.cs = csp.tile([P, 512], F32, tag="cs")
                    for t in self.vt:
                        nc.vector.memset(t[:], 0.0)
                    nc.vector.memset(self.cs[:], 0.0)
                    self.ets = {}

                def emit_scores_exp(self, kt):
                    g, qoff = self.g, self.qoff
                    for hp, pb in ((0, 0), (1, 64)):
                        ps_s = psp.tile([P, 1024], F32, tag="ps", name=f"ps_s{hp}")
                        for qb in range(2):
                            nc.tensor.matmul(
                                ps_s[:, qb * 512 : (qb + 1) * 512],
                                lhsT=KT[g][pb : pb + 64, kt * P : (kt + 1) * P],
                                rhs=QT[g][
                                    pb : pb + 64,
                                    qoff + qb * 512 : qoff + (qb + 1) * 512,
                                ],
                                start=True,
                                stop=True,
                            )
                        et = expp.tile([P, 1024], BF16, tag="expT", name=f"et{hp}")
                        nc.scalar.activation(
                            et[:], ps_s[:], mybir.ActivationFunctionType.Exp
                        )
                        self.ets[(kt, hp)] = et

                def emit_v_cs(self, kt):
                    g = self.g
                    last = kt == NKT - 1
                    et = {hp: self.ets.pop((kt, hp)) for hp in (0, 1)}
                    for qb in range(2):
                        for hp, pb in ((0, 0), (1, 64)):
                            nc.tensor.matmul(
                                self.vt[qb][pb : pb + 64, :],
                                lhsT=V_st[kt][:, 2 * g + hp, 0:DK],
                                rhs=et[hp][:, qb * 512 : (qb + 1) * 512],
                                start=False,
                                stop=last,
                                skip_group_check=True,
                                tile_position=(0, pb),
                            )
                    for hp in (0, 1):
                        for qb in range(2):
                            cp = 64 * hp + 32 * qb
                            nc.tensor.matmul(
                                self.cs[cp : cp + 1, :],
                                lhsT=ones_bf[:],
                                rhs=et[hp][:, qb * 512 : (qb + 1) * 512],
                                start=False,
                                stop=last,
                                skip_group_check=True,
                                tile_position=(0, cp),
                            )
                    if last:
                        self.emit_norm()

                def emit_norm(self):
                    g, qoff = self.g, self.qoff
                    un = [
                        rcp.tile([P, 512], F32, tag=f"un{qb}", name=f"un{qb}")
                        for qb in range(2)
                    ]
                    for qb in range(2):
                        nc.vector.tensor_copy(out=un[qb][:], in_=self.vt[qb][:])
                    cs_sb = rcp.tile([P, 512], F32, tag="cs_sb")
                    nc.vector.tensor_copy(out=cs_sb[:], in_=self.cs[:])
                    # reciprocal of the whole tile; junk rows give inf but only
                    # the four real rows {0,32,64,96} are read (by the DMA)
                    cs_rc = rcp.tile([P, 512], F32, tag="cs_rc")
                    nc.vector.reciprocal(cs_rc[:], cs_sb[:])
                    zd = dramp.tile([4, 512], F32, name="zd")
                    # zd rows: 0=(A,qb0) 1=(A,qb1) 2=(B,qb0) 3=(B,qb1)
                    nc.sync.dma_start(zd[:], cs_rc[0:128:32, :])
                    for qb in range(2):
                        rcb = rcp.tile(
                            [P, 512], F32, tag=f"rcb{qb}", name=f"rcb{qb}"
                        )
                        nc.sync.dma_start(
                            rcb[0:64, :], zd[qb, None, :].to_broadcast([64, 512])
                        )
                        nc.sync.dma_start(
                            rcb[64:128, :],
                            zd[qb + 2, None, :].to_broadcast([64, 512]),
                        )
                        for pb in (0, 64):
                            nc.vector.tensor_mul(
                                out=attn[g][
                                    pb : pb + 64,
                                    qoff + qb * 512 : qoff + (qb + 1) * 512,
                                ],
                                in0=un[qb][pb : pb + 64, :],
                                in1=rcb[pb : pb + 64, :],
                            )

            pending = None
            gkt = 0
            for g in range(NG):
                for qh in range(2):
                    blk = AttnBlock(g, qh)
                    for kt in range(NKT):
                        blk.emit_scores_exp(kt)
                        if pending is not None:
                            pending[0].emit_v_cs(pending[1])
                        pending = (blk, kt)
                        injector.tick()
                        gkt += 1
            pending[0].emit_v_cs(pending[1])
            injector.drain()

            # ---------------- output projection (second half; the first half
            # was injected into the last attention block's PE slack)
            for st in range(NKT):
                for ob in range(2):
                    wo_chunk(st, ob, psp, "ps")

    _split_sync_waits(nc)
    return nc


_NC = None


def _get_nc():
    global _NC
    if _NC is None:
        _NC = build_nc()
    return _NC


# ---------------------------------------------------------------- host side
def make_in_maps(x, wq, wk, wv, wo):
    x = np.asarray(x, dtype=np.float32)
    wq = np.asarray(wq, dtype=np.float32)
    wk = np.asarray(wk, dtype=np.float32)
    wv = np.asarray(wv, dtype=np.float32)
    wo = np.asarray(wo, dtype=np.float32)
    in_maps = []
    for c in range(N_CORES):
        b, hg = c // 2, c % 2
        sl = slice(hg * DL, (hg + 1) * DL)
        xTc = np.ascontiguousarray(x[b].T).astype(BF16_NP)
        wqTc = np.ascontiguousarray((wq[sl] / 8.0).T).astype(BF16_NP)
        wkTc = np.ascontiguousarray(wk[sl].T).astype(BF16_NP)
        wvTc = np.ascontiguousarray(wv[sl].T).astype(BF16_NP)
        woTc = np.ascontiguousarray(wo[:, sl].T).astype(BF16_NP)
        in_maps.append(
            {"xT": xTc, "wqT": wqTc, "wkT": wkTc, "wvT": wvTc, "woT": woTc}
        )
    return in_maps


def gather(results):
    out = np.zeros((4, S, DM), dtype=np.float32)
    for c in range(N_CORES):
        out[c // 2] += results[c]["out"]
    return out


def kernel(x, wq, wk, wv, wo):
    from concourse.bass_utils import run_bass_kernel_spmd

    nc = _get_nc()
    in_maps = make_in_maps(x, wq, wk, wv, wo)
    res = run_bass_kernel_spmd(nc, in_maps, CORE_IDS)
    return gather(res.results)
